# revision 1
# baseline (speedup 1.0000x reference)
"""nn_Attention4 kernel: embedding (max-norm renorm) -> bi-GRU -> ragged span
mean-pool -> attention -> linear head.

Primary path executes on the Trainium NeuronCores via JAX (axon/PJRT backend);
a NumPy fallback guarantees a correct result if the device path is unavailable.
"""
import signal
import numpy as np

EMBED_NUM = 50000
EMBED_DIM = 300
HIDDEN = 512
ATT = 256
LABELS = 3
B, S = 64, 256
MAX_NORM = 5.0

_ORDER = ("emb", "Wih_f", "Whh_f", "bih_f", "bhh_f", "Wih_b", "Whh_b",
          "bih_b", "bhh_b", "W1", "b1", "u", "W2", "b2")


# ---------------------------------------------------------------- jax path
def _kernel_jax(x, target_start, target_end, **w):
    import jax
    import jax.numpy as jnp

    def gru_dir(xw, Whh, bhh):
        def step(h, xt):
            gh = h @ Whh.T + bhh
            xr, xz, xn = jnp.split(xt, 3, axis=-1)
            hr, hz, hn = jnp.split(gh, 3, axis=-1)
            r = jax.nn.sigmoid(xr + hr)
            z = jax.nn.sigmoid(xz + hz)
            n = jnp.tanh(xn + r * hn)
            h = (1.0 - z) * n + z * h
            return h, h

        h0 = jnp.zeros((xw.shape[0], HIDDEN), dtype=xw.dtype)
        _, hs = jax.lax.scan(step, h0, jnp.swapaxes(xw, 0, 1))
        return jnp.swapaxes(hs, 0, 1)

    x = jnp.asarray(np.asarray(x).astype(np.int32))
    target_start = jnp.asarray(np.asarray(target_start).astype(np.int32))
    target_end = jnp.asarray(np.asarray(target_end).astype(np.int32))
    (emb, Wih_f, Whh_f, bih_f, bhh_f, Wih_b, Whh_b, bih_b, bhh_b,
     W1, b1, u, W2, b2) = [jnp.asarray(np.asarray(w[k], np.float32))
                           for k in _ORDER]

    e = emb[x]
    nrm = jnp.linalg.norm(e, axis=-1, keepdims=True)
    e = e * jnp.minimum(1.0, MAX_NORM / (nrm + 1e-7))

    h_f = gru_dir(e @ Wih_f.T + bih_f, Whh_f, bhh_f)
    h_b = gru_dir(e[:, ::-1, :] @ Wih_b.T + bih_b, Whh_b, bhh_b)[:, ::-1, :]
    h = jnp.concatenate([h_f, h_b], axis=-1)

    t = jnp.arange(S)
    mask = (t[None, :] >= target_start[:, None]) & (t[None, :] <= target_end[:, None])
    cnt = (target_end - target_start + 1).astype(h.dtype)
    target = (h * mask[..., None].astype(h.dtype)).sum(axis=1) / cnt[:, None]

    cat = jnp.concatenate([h, jnp.broadcast_to(target[:, None, :], h.shape)],
                          axis=-1)
    o = jnp.tanh(cat @ W1.T + b1)

    beta = jnp.einsum("ka,bsa->bks", u, o)
    alfa = jax.nn.softmax(beta, axis=-1)
    result = jnp.einsum("bks,bsh->bkh", alfa, h)
    out = result @ W2.T + b2
    return np.asarray(out, dtype=np.float32)


# -------------------------------------------------------------- numpy path
def _sigmoid(v):
    return 1.0 / (1.0 + np.exp(-v))


def _gru_np(xw, Whh, bhh):
    b = xw.shape[0]
    h = np.zeros((b, HIDDEN), np.float32)
    hs = np.empty((b, S, HIDDEN), np.float32)
    WhhT = np.ascontiguousarray(Whh.T)
    for t in range(S):
        gh = h @ WhhT + bhh
        xr, xz, xn = np.split(xw[:, t, :], 3, axis=-1)
        hr, hz, hn = np.split(gh, 3, axis=-1)
        r = _sigmoid(xr + hr)
        z = _sigmoid(xz + hz)
        n = np.tanh(xn + r * hn)
        h = (1.0 - z) * n + z * h
        hs[:, t, :] = h
    return hs


def _kernel_numpy(x, target_start, target_end, **w):
    x = np.asarray(x).astype(np.int64)
    target_start = np.asarray(target_start).astype(np.int64)
    target_end = np.asarray(target_end).astype(np.int64)
    (emb, Wih_f, Whh_f, bih_f, bhh_f, Wih_b, Whh_b, bih_b, bhh_b,
     W1, b1, u, W2, b2) = [np.asarray(w[k], np.float32) for k in _ORDER]

    e = emb[x]
    nrm = np.linalg.norm(e, axis=-1, keepdims=True)
    e = e * np.minimum(1.0, MAX_NORM / (nrm + 1e-7))

    h_f = _gru_np(e @ Wih_f.T + bih_f, Whh_f, bhh_f)
    h_b = _gru_np(e[:, ::-1, :] @ Wih_b.T + bih_b, Whh_b, bhh_b)[:, ::-1, :]
    h = np.concatenate([h_f, h_b], axis=-1)

    t = np.arange(S)
    mask = (t[None, :] >= target_start[:, None]) & (t[None, :] <= target_end[:, None])
    cnt = (target_end - target_start + 1).astype(h.dtype)
    target = (h * mask[..., None].astype(h.dtype)).sum(axis=1) / cnt[:, None]

    cat = np.concatenate([h, np.broadcast_to(target[:, None, :], h.shape)],
                         axis=-1)
    o = np.tanh(cat @ W1.T + b1)

    beta = np.einsum("ka,bsa->bks", u, o)
    beta -= beta.max(axis=-1, keepdims=True)
    ez = np.exp(beta)
    alfa = ez / ez.sum(axis=-1, keepdims=True)
    result = np.einsum("bks,bsh->bkh", alfa, h)
    return (result @ W2.T + b2).astype(np.float32)


class _Timeout(Exception):
    pass


def kernel(**inputs):
    # Device path with a hard timeout guard; any failure falls back to the
    # (verified) host implementation so a wedged accelerator can't hang us.
    try:
        def _raise(signum, frame):
            raise _Timeout()

        old = None
        try:
            old = signal.signal(signal.SIGALRM, _raise)
            signal.alarm(600)
        except ValueError:
            old = None  # not in main thread; run unguarded
        try:
            return _kernel_jax(**inputs)
        finally:
            try:
                signal.alarm(0)
                if old is not None:
                    signal.signal(signal.SIGALRM, old)
            except ValueError:
                pass
    except BaseException:
        return _kernel_numpy(**inputs)

